# revision 14
# baseline (speedup 1.0000x reference)
"""FP8 batch-matmul-dense kernel for Trainium2 (8 NeuronCores, batch-sharded).

Problem: out[b] = fp8qdq(x)[b] @ fp8qdq(w)[b] + bias[b]
  x: [32, 512, 2048] f32, w: [32, 2048, 2048] f32, bias: [32, 1, 2048] f32
  fp8qdq = torchao-style dynamic tensorwise scaling: s = 448/amax(|t|),
  q = e4m3fn(t*s), dq = q/s. Global (whole-tensor) amax.

Sharding: batch axis across 8 cores, 4 slices each (expert-parallel style).

Two NEFFs:
  A) per-core amax of |x| and |w| shards (DVE absmax reduce), host maxes the
     8x2 scalars (exact: max is order-independent).
  B) quantize + batched matmul. Host computes scales; we use s' = 224/amax
     (= fl(448/amax)/2 exactly) because TRN fp8_e4m3 tops out at 240, not
     448: the OCP e4m3fn lattice scaled by 1/2 lands exactly on the TRN
     lattice (mismatch only in the sub-subnormal tail, ~1e-5 absolute).
     Matmul runs on the raw fp8 codes (exact products, fp32 PSUM accum) and
     the output is rescaled by c = 1/(sx'*sw').
"""

import os
import sys

for _p in ("/root/.axon_site", "/root/.axon_site/_ro/trn_rl_repo", "/opt/trn_rl_repo"):
    if os.path.isdir(_p) and _p not in sys.path:
        sys.path.append(_p)

import numpy as np

import concourse.bass as bass
import concourse.bass_isa as bass_isa
import concourse.mybir as mybir
import concourse.tile as tile
from concourse import bacc
from concourse.bass_utils import run_bass_kernel_spmd
from concourse.masks import make_identity

# Problem shape (hardcoded per contest rules).
B, M, K, N = 32, 512, 2048, 2048
NCORES = 8
BL = B // NCORES          # 4 batch slices per core
P = 128
KT = K // P               # 16 k-tiles
MT = M // P               # 4 m-tiles
NFREE = 512               # matmul moving free dim (one PSUM bank)
NT = N // NFREE           # 4 n-tiles
FP8_HALF_MAX = np.float32(224.0)  # 448/2: OCP grid mapped onto TRN e4m3

F32 = mybir.dt.float32
FP8 = mybir.dt.float8e4

_cache = {}


def _build_amax_nc():
    nc = bacc.Bacc("TRN2", target_bir_lowering=False, debug=False, num_devices=NCORES)
    x = nc.dram_tensor("x", [BL, M, K], F32, kind="ExternalInput")
    w = nc.dram_tensor("w", [BL, K, N], F32, kind="ExternalInput")
    amax_out = nc.dram_tensor("amax_out", [1, 2], F32, kind="ExternalOutput")

    n_x = BL * MT // 4       # 4 tiles of [128, 4, 2048] (4 MiB DMAs)
    n_w = BL * KT // 4       # 16 tiles of [128, 4, 2048]

    with tile.TileContext(nc) as tc:
        with (
            tc.tile_pool(name="stage", bufs=4) as stage,
            tc.tile_pool(name="acc", bufs=1) as accp,
        ):
            acc = accp.tile([P, n_x + n_w], F32, name="acc")
            col = 0
            for b in range(BL):
                for mt in range(0, MT, 4):
                    t = stage.tile([P, 4, K], F32, name="xs", tag="stage")
                    src = x[b, mt * P:(mt + 4) * P, :].rearrange(
                        "(p k) n -> k p n", p=4
                    )
                    nc.sync.dma_start(t[:], src)
                    nc.vector.tensor_reduce(
                        acc[:, col:col + 1], t[:],
                        axis=mybir.AxisListType.XY, op=mybir.AluOpType.max,
                        apply_absolute_value=True,
                    )
                    col += 1
            for b in range(BL):
                for kt in range(0, KT, 4):
                    t = stage.tile([P, 4, N], F32, name="ws", tag="stage")
                    src = w[b, kt * P:(kt + 4) * P, :].rearrange(
                        "(p k) n -> k p n", p=4
                    )
                    nc.sync.dma_start(t[:], src)
                    nc.vector.tensor_reduce(
                        acc[:, col:col + 1], t[:],
                        axis=mybir.AxisListType.XY, op=mybir.AluOpType.max,
                        apply_absolute_value=True,
                    )
                    col += 1

            red = accp.tile([P, 2], F32, name="red")
            nc.vector.tensor_reduce(
                red[:, 0:1], acc[:, 0:n_x],
                axis=mybir.AxisListType.X, op=mybir.AluOpType.max,
            )
            nc.vector.tensor_reduce(
                red[:, 1:2], acc[:, n_x:n_x + n_w],
                axis=mybir.AxisListType.X, op=mybir.AluOpType.max,
            )
            par = accp.tile([P, 2], F32, name="par")
            nc.gpsimd.partition_all_reduce(
                par[:], red[:], channels=P, reduce_op=bass_isa.ReduceOp.max
            )
            nc.sync.dma_start(amax_out[0:1, :], par[0:1, :])

    nc.compile()
    return nc


def _build_mm_nc():
    nc = bacc.Bacc("TRN2", target_bir_lowering=False, debug=False, num_devices=NCORES)
    x = nc.dram_tensor("x", [BL, M, K], F32, kind="ExternalInput")
    w = nc.dram_tensor("w", [BL, K, N], F32, kind="ExternalInput")
    bias = nc.dram_tensor("bias", [BL, 1, N], F32, kind="ExternalInput")
    consts = nc.dram_tensor("consts", [1, 4], F32, kind="ExternalInput")
    out = nc.dram_tensor("out", [BL, M, N], F32, kind="ExternalOutput")

    with tile.TileContext(nc) as tc:
        with (
            tc.tile_pool(name="small", bufs=1) as small,
            tc.tile_pool(name="bias1", bufs=2) as bias1p,
            tc.tile_pool(name="biasb", bufs=2) as biasbp,
            tc.tile_pool(name="xstage", bufs=3) as xstage,  # [P,2,K] 2MB tiles
            tc.tile_pool(name="xqt", bufs=2) as xqtp,
            tc.tile_pool(name="wstage", bufs=3) as wstage,
            tc.tile_pool(name="wq", bufs=12) as wqp,
            tc.tile_pool(name="ostage", bufs=2) as ostage,
            tc.tile_pool(name="trps", bufs=2, space="PSUM") as trps,
            tc.tile_pool(name="mmps", bufs=6, space="PSUM") as mmps,
        ):
            ident = small.tile([P, P], F32, name="ident")
            make_identity(nc, ident[:])

            cst = small.tile([1, 4], F32, name="cst")
            nc.sync.dma_start(cst[:], consts[0:1, :])
            cb = small.tile([P, 4], F32, name="cb")
            nc.gpsimd.partition_broadcast(cb[:], cst[:])
            sx_ap = cb[:, 0:1]   # 224/amax_x
            sw_ap = cb[:, 1:2]   # 224/amax_w
            c_ap = cb[:, 2:3]    # 1/(sx*sw)

            for b in range(BL):
                b1 = bias1p.tile([1, N], F32, name="b1", tag="b1")
                nc.sync.dma_start(b1[:], bias[b, :, :])
                bb = biasbp.tile([P, N], F32, name="bb", tag="bb")
                nc.gpsimd.partition_broadcast(bb[:], b1[:])

                # x: load m-row tiles, PE-transpose 128x128 blocks, quantize
                # (scale sx, cast fp8) straight out of PSUM into resident xqT.
                xq = xqtp.tile([P, KT, M], FP8, name="xq", tag="xq")
                for mh in range(2):
                    xs = xstage.tile([P, 2, K], F32, name="xs", tag="xs")
                    src = x[b, mh * 2 * P:(mh * 2 + 2) * P, :].rearrange(
                        "(p k) n -> k p n", p=2
                    )
                    nc.sync.dma_start(xs[:], src)
                    for par in range(2):
                        mt = mh * 2 + par
                        for kt in range(KT):
                            ps = trps.tile([P, P], F32, name="tps", tag="tps")
                            nc.tensor.transpose(
                                ps[:], xs[:, par, kt * P:(kt + 1) * P], ident[:]
                            )
                            # quant-copy on DVE (2 elem/cyc) keeps the
                            # transpose chain moving; ACT does w-quant.
                            nc.vector.tensor_scalar(
                                xq[:, kt, mt * P:(mt + 1) * P], ps[:], sx_ap, None,
                                op0=mybir.AluOpType.mult,
                            )

                # w: stream paired k-row tiles [128, 2, N] (2 MiB DMAs, and
                # exactly the DoubleRow rhs layout: parity axis = two
                # consecutive 128-row blocks), quantize to resident fp8
                # (split between ACT and DVE to balance engine time).
                wq_tiles = []
                for t in range(KT // 2):
                    wqt = wqp.tile([P, 2, N], FP8, name="wq", tag="wq")
                    for par in range(2):
                        ws = wstage.tile([P, N], F32, name="ws", tag="ws")
                        kt = 2 * t + par
                        nc.sync.dma_start(ws[:], w[b, kt * P:(kt + 1) * P, :])
                        nc.scalar.activation(
                            wqt[:, par, :], ws[:],
                            mybir.ActivationFunctionType.Copy, scale=sw_ap,
                        )
                    wq_tiles.append(wqt)

                for mt in range(MT):
                    if mt % 2 == 0:
                        ost2 = ostage.tile([P, 2, N], F32, name="ost", tag="ost")
                    ost = ost2[:, mt % 2, :]
                    psums = [
                        mmps.tile([P, NFREE], F32, name=f"mmps{nt}", tag="mmps")
                        for nt in range(NT)
                    ]
                    for t in range(KT // 2):
                        # [128, 2, 128]: k = (2t+parity)*128 + ki, matching
                        # the rhs pairing below.
                        lhsT = xq[:, 2 * t:2 * t + 2, mt * P:(mt + 1) * P]
                        for nt in range(NT):
                            nc.tensor.matmul(
                                psums[nt][:],
                                lhsT,
                                wq_tiles[t][:, :, nt * NFREE:(nt + 1) * NFREE],
                                start=(t == 0),
                                stop=(t == KT // 2 - 1),
                                perf_mode=mybir.MatmulPerfMode.DoubleRow,
                            )
                    for nt in range(NT):
                        nc.vector.scalar_tensor_tensor(
                            ost[:, nt * NFREE:(nt + 1) * NFREE],
                            psums[nt][:],
                            c_ap,
                            bb[:, nt * NFREE:(nt + 1) * NFREE],
                            op0=mybir.AluOpType.mult,
                            op1=mybir.AluOpType.add,
                        )
                    # SWDGE store: keeps the in-order HWDGE (sync) queue free
                    # for next-slice loads — out stores wait on matmuls and
                    # would otherwise stall b+1's weight prefetch.
                    if mt % 2 == 1:
                        nc.gpsimd.dma_start(
                            out[b, (mt - 1) * P:(mt + 1) * P, :].rearrange(
                                "(p k) n -> k p n", p=2
                            ),
                            ost2[:],
                        )

    nc.compile()
    return nc


def _get_nc(name):
    if name not in _cache:
        _cache[name] = _build_amax_nc() if name == "amax" else _build_mm_nc()
    return _cache[name]


# test.py introspection: exec times (ns) of the last kernel() call.
last_run_info = {}


def kernel(input, weight, bias, _profile=False, _trace_kwargs=None):
    input = np.ascontiguousarray(input, dtype=np.float32)
    weight = np.ascontiguousarray(weight, dtype=np.float32)
    bias = np.ascontiguousarray(bias, dtype=np.float32)
    assert input.shape == (B, M, K) and weight.shape == (B, K, N)
    assert bias.shape == (B, 1, N)

    x_sh = [input[c * BL:(c + 1) * BL] for c in range(NCORES)]
    w_sh = [weight[c * BL:(c + 1) * BL] for c in range(NCORES)]
    b_sh = [bias[c * BL:(c + 1) * BL] for c in range(NCORES)]

    core_ids = list(range(NCORES))
    kw = dict(trace=_profile)
    if _trace_kwargs:
        kw.update(_trace_kwargs)

    # --- NEFF A: per-shard amax ---
    nc_a = _get_nc("amax")
    res_a = run_bass_kernel_spmd(
        nc_a,
        [{"x": x_sh[c], "w": w_sh[c]} for c in range(NCORES)],
        core_ids=core_ids,
        **kw,
    )
    amaxes = np.stack([res_a.results[c]["amax_out"][0] for c in range(NCORES)])
    ax = np.float32(max(amaxes[:, 0].max(), np.float32(1e-12)))
    aw = np.float32(max(amaxes[:, 1].max(), np.float32(1e-12)))

    # Exact: fl(224/a) == fl(448/a)/2 (power-of-two scaling commutes w/ RNE).
    sx = FP8_HALF_MAX / ax
    sw = FP8_HALF_MAX / aw
    c_out = np.float32(1.0 / (np.float64(sx) * np.float64(sw)))
    consts = np.array([[sx, sw, c_out, 0.0]], dtype=np.float32)

    # --- NEFF B: quantize + batched matmul ---
    nc_b = _get_nc("mm")
    res_b = run_bass_kernel_spmd(
        nc_b,
        [
            {"x": x_sh[c], "w": w_sh[c], "bias": b_sh[c], "consts": consts}
            for c in range(NCORES)
        ],
        core_ids=core_ids,
        **kw,
    )

    last_run_info.clear()
    last_run_info["amax_exec_ns"] = res_a.exec_time_ns
    last_run_info["mm_exec_ns"] = res_b.exec_time_ns
    last_run_info["amax_results"] = res_a
    last_run_info["mm_results"] = res_b

    out = np.concatenate(
        [res_b.results[c]["out"] for c in range(NCORES)], axis=0
    )
    return out
